# revision 1
# baseline (speedup 1.0000x reference)
"""GAT (2-layer graph attention network) Trainium2 Bass kernel, 8-core SPMD.

Sharding: core c owns output rows i in [c*512, (c+1)*512) for BOTH layers and
computes ALL 8 heads of layer 1 for those rows (column/row-parallel instead of
head-parallel). Wins vs head-parallel: the adjacency stripe adjT[:, slice] is
loaded once (4.2MB fp16) and reused by all 8 heads AND layer 2 (8x less HBM
traffic than one full adj per core), and the inter-layer exchange collapses to
a single AllGather of h @ Wo_ext [4096, 66] fp16 (no ReduceScatter at all,
since each core holds complete h rows).

Math: with s_ij = f_src[i] + f_dst[j], exp(lrelu(s)) equals, up to a per-i
factor that cancels in softmax, max(g[i], r[j]) * e1[j] with
g = exp(0.8*f_src), r = exp(-0.8*f_dst), e1 = exp(f_dst - C). So the masked
unnormalized score matrix is M[j,i] = adj[j,i] * (g[i] max r[j]) * e1[j]
against a RAW lhsT [Wh | 1]. (g max r)*e1 is ONE dual-op DVE tensor_scalar
(4x fp16 mode, both scalars per-partition [128,1]), and the adjacency mask is
one tensor_tensor mult over a 4-j-tile-wide quad (2x mode), split between DVE
and Pool (TensorTensor is Pool's only legal elementwise op on TRN2 silicon;
anything TensorScalarPtr-shaped must stay on DVE).

kernel(**inputs) takes full unsharded inputs, returns the full output.
"""

from contextlib import ExitStack

import numpy as np

import concourse.mybir as mybir
import concourse.tile as tile
from concourse import bacc
from concourse.bass_utils import run_bass_kernel_spmd

# Steer every activation to the one ACT table set covering all functions this
# kernel uses (Exp, Identity, Ln, Copy) so no mid-kernel table reloads happen.
_orig_get_tables = bacc.get_activation_tables


def _pinned_tables(arch):
    tabs = _orig_get_tables(arch)
    if "natural_log_exp_and_others" in tabs:
        return {name: (funcs if name == "natural_log_exp_and_others" else set())
                for name, funcs in tabs.items()}
    return tabs


bacc.get_activation_tables = _pinned_tables

N = 4096
F = 512
D = 64          # per-head hidden == n classes
H = 8
P = 128
NT = N // P     # 32 j tiles
SL = 512        # i columns per core
NKF = F // P    # 4 contraction tiles for x @ W
NQ = 8          # j quads (4 j-tiles each)
C_DST = 7.0      # layer-1 exponent shift: keeps u = max(g,r)*e1 under fp16 max
N_CORES = 8

F32 = mybir.dt.float32
F16 = mybir.dt.float16
A = mybir.AluOpType
AF = mybir.ActivationFunctionType

# (h, q) layer-1 quads whose adjacency mask-multiply runs on the Pool engine.
# Spread evenly over all heads; within each head Pool quads are interleaved
# early-ish (never last) so the Pool engine overlaps the DVE quads without
# the accumulation chain stalling on a leading slow Pool op.
N_POOL_L1 = 30
POOL_L1 = {(i * 64 // N_POOL_L1) for i in range(N_POOL_L1)} if N_POOL_L1 else set()
POOL_L2 = {6, 7}    # layer-2 quads whose mask-multiply runs on Pool

_CACHED = {}


def build_kernel():
    nc = bacc.Bacc("TRN2", num_devices=N_CORES)

    xT = nc.dram_tensor("xT", [F, N], F16, kind="ExternalInput")
    xS = nc.dram_tensor("xS", [F, SL], F16, kind="ExternalInput")
    adjQ = nc.dram_tensor("adjQ", [NQ * P, 4 * SL], F16, kind="ExternalInput")
    Wext = nc.dram_tensor("Wext", [F, H * 66], F16, kind="ExternalInput")
    selD = nc.dram_tensor("selD", [H, H * P], F16, kind="ExternalInput")
    Woext = nc.dram_tensor("Woext", [F, 66], F16, kind="ExternalInput")
    outT = nc.dram_tensor("outT", [D, SL], F32, kind="ExternalOutput")

    with ExitStack() as ctx:
        tc = ctx.enter_context(tile.TileContext(nc))
        psum = ctx.enter_context(tc.tile_pool(name="psum", bufs=1, space="PSUM"))
        persist = ctx.enter_context(tc.tile_pool(name="persist", bufs=1))
        work = ctx.enter_context(tc.tile_pool(name="work", bufs=1))
        dram = ctx.enter_context(tc.tile_pool(name="dram", bufs=1, space="DRAM"))

        ones1 = persist.tile([1, P], F32, tag="ones1")
        nc.vector.memset(ones1[:], 1.0)
        bias_c = persist.tile([P, 1], F32, tag="bias_c")
        nc.vector.memset(bias_c[:], -C_DST)

        # ---- input DMAs (sel first: tiny, and emit_g head-of-line blocks
        # the PE queue on it) ---------------------------------------------
        sel = persist.tile([H, H * P], F16, tag="sel", name="sel")
        nc.sync.dma_start(out=sel[:], in_=selD[:])
        xs_sb = []
        for kf in range(NKF):
            t = persist.tile([P, SL], F16, tag=f"xs{kf}", name=f"xs{kf}")
            nc.sync.dma_start(out=t[:], in_=xS[kf * P:(kf + 1) * P, :])
            xs_sb.append(t)
        wext_sb = []
        for kf in range(NKF):
            t = persist.tile([P, H * 66], F16, tag=f"we{kf}", name=f"we{kf}")
            nc.sync.dma_start(out=t[:], in_=Wext[kf * P:(kf + 1) * P, :])
            wext_sb.append(t)
        xt_sb = [persist.tile([P, N], F16, tag=f"xt{kf}", name=f"xt{kf}")
                 for kf in range(NKF)]
        adjq_sb = [persist.tile([P, 4 * SL], F16, tag=f"adjq{q}",
                                name=f"adjq{q}") for q in range(NQ)]
        # interleave xt column-blocks with the adj quads they unblock, so
        # attention on early quads starts while later inputs still stream
        for b in range(4):
            for kf in range(NKF):
                nc.sync.dma_start(
                    out=xt_sb[kf][:, b * 1024:(b + 1) * 1024],
                    in_=xT[kf * P:(kf + 1) * P, b * 1024:(b + 1) * 1024])
            for q in (2 * b, 2 * b + 1):
                nc.sync.dma_start(out=adjq_sb[q][:],
                                  in_=adjQ[q * P:(q + 1) * P, :])
        woext_sb = []
        for kt in range(NKF):
            t = persist.tile([P, 66], F16, tag=f"wo{kt}", name=f"wo{kt}")
            nc.sync.dma_start(out=t[:], in_=Woext[kt * P:(kt + 1) * P, :])
            woext_sb.append(t)

        # ---- f_src rows for all 8 heads in ONE matmul chain -------------
        # lhsT = strided f_src weight columns [128, 8] -> fr_ps [8, 512]
        fr_ps = psum.tile([H, SL], F32, tag="bank", bufs=4, name="fr")
        for kf in range(NKF):
            nc.tensor.matmul(fr_ps[:], wext_sb[kf][:, 64:528:66],
                             xs_sb[kf][:], start=(kf == 0),
                             stop=(kf == NKF - 1))
        fsr = persist.tile([H, SL], F16, tag="fsr", name="fsr")
        nc.scalar.activation(fsr[:], fr_ps[:], AF.Copy)
        # sel: one-hot selector tiles, sel[k, h*128+p] = (k == h), so a K=8
        # matmul against the full fsr broadcasts row h down 128 partitions
        # without a partition-offset rhs (which BIR rejects). Host-provided.
        g_bc = [persist.tile([P, SL], F16, tag=f"g{h}", name=f"g{h}")
                for h in range(H)]

        def emit_g(h):
            # broadcast row h of fsr down 128 partitions, then exp(0.8 x)
            bc_ps = psum.tile([P, SL], F32, tag="bank", bufs=4, name=f"gb{h}")
            nc.tensor.matmul(bc_ps[:], sel[:, h * P:(h + 1) * P], fsr[:],
                             start=True, stop=True)
            nc.scalar.activation(g_bc[h][:], bc_ps[:], AF.Exp, scale=0.8)

        # ---- stage prep: Wh_ext tiles (4 heads wide), e1/r, ones col ----
        # stage[t][qd] = fp16 [Wh_h|f_src_h|f_dst_h for h in 4qd..4qd+3],
        # then col 64 of each head slot is overwritten with 1.0 (the
        # denominator column); f_dst (col 65) stays for e1/r extraction.
        stage = [persist.tile([P, 528], F16, tag=f"st{t}", name=f"st{t}")
                 for t in range(NT)]
        e1q = [persist.tile([P, H], F32, tag=f"e1_{t}", name=f"e1_{t}")
               for t in range(NT)]
        rq = [persist.tile([P, H], F32, tag=f"r_{t}", name=f"r_{t}")
              for t in range(NT)]

        def emit_stage_half(t, qd):
            """Wh_ext matmuls + fp16 staging + e1/r + ones col for heads
            [4qd, 4qd+4) at j-tile t, plus the Pool-tile e1 pre-folds."""
            st = stage[t]
            wh_ps = psum.tile([P, 264], F32, tag="bank", bufs=4,
                              name=f"wh{t}_{qd}")
            for kf in range(NKF):
                nc.tensor.matmul(
                    wh_ps[:], xt_sb[kf][:, t * P:(t + 1) * P],
                    wext_sb[kf][:, qd * 264:(qd + 1) * 264],
                    start=(kf == 0), stop=(kf == NKF - 1))
            # qd=0 staging copies split DVE/ACT (both engines have slack in
            # the prep phase); qd=1 copies all on ACT (DVE is saturated by
            # attention by then)
            dst = st[:, qd * 264:(qd + 1) * 264]
            if qd == 0 and t % 2 == 0:
                nc.vector.tensor_copy(dst, wh_ps[:])
            else:
                nc.scalar.activation(dst, wh_ps[:], AF.Copy)
            lo = qd * 264
            nc.scalar.activation(e1q[t][:, 4 * qd:4 * qd + 4],
                                 st[:, lo + 65:lo + 264:66], AF.Exp,
                                 bias=bias_c[:])
            nc.scalar.activation(rq[t][:, 4 * qd:4 * qd + 4],
                                 st[:, lo + 65:lo + 264:66], AF.Exp,
                                 scale=-0.8)
            nc.vector.memset(st[:, lo + 64:lo + 264:66], 1.0)

        # heads 0-3 stages first (with g broadcasts interleaved); heads 4-7
        # stages are emitted between the first heads' attention chains below,
        # filling PE gaps and avoiding a long serial prep phase.
        for t in range(NT):
            if t < H:
                emit_g(t)
            emit_stage_half(t, 0)

        def lhst(h, jt):
            return stage[jt][:, h * 66:h * 66 + 65]

        # ---- layer-1 attention: h outer, quad q inner -------------------
        hT = [persist.tile([P, SL], F16, tag=f"hT{kt}", name=f"hT{kt}")
              for kt in range(NKF)]
        # Wh2 = h @ Wo_ext and the f_src2 row, accumulated one kt (head pair)
        # at a time right after that pair's hT half is written, so only the
        # last kt's small matmuls sit on the pre-AllGather critical path
        wh2acc = persist.tile([P, 264], F32, tag="wh2acc", name="wh2acc")
        fr2acc = persist.tile([1, SL], F32, tag="fr2acc", name="fr2acc")

        def emit_wh2_part(kt):
            o2_ps = psum.tile([P, 264], F32, tag="bank", bufs=4,
                              name=f"o2p{kt}")
            for sub in range(4):
                nc.tensor.matmul(o2_ps[:, sub * 66:(sub + 1) * 66],
                                 hT[kt][:, sub * P:(sub + 1) * P],
                                 woext_sb[kt][:], start=True, stop=True)
            fr2_ps = psum.tile([1, SL], F32, tag="bank", bufs=4,
                               name=f"fr2p{kt}")
            nc.tensor.matmul(fr2_ps[:], woext_sb[kt][:, 64:65], hT[kt][:],
                             start=True, stop=True)
            if kt == 0:
                nc.scalar.activation(wh2acc[:], o2_ps[:], AF.Copy)
                nc.scalar.activation(fr2acc[:], fr2_ps[:], AF.Copy)
            else:
                nc.vector.tensor_add(wh2acc[:], wh2acc[:], o2_ps[:])
                nc.vector.tensor_add(fr2acc[:], fr2acc[:], fr2_ps[:])

        for h in range(H):
            if h < 4:
                for t in range(h * 8, h * 8 + 8):
                    emit_stage_half(t, 1)
            if h % 2 == 1 and h > 1:
                emit_wh2_part(h // 2 - 1)
            acc = psum.tile([D + 1, SL], F32, tag="acc", bufs=4,
                            name=f"acc{h}")
            pq = [q for q in range(NQ) if (h * 8 + q) in POOL_L1]
            dq = [q for q in range(NQ) if (h * 8 + q) not in POOL_L1]
            # one DVE quad leads (fast first matmuls), then alternate Pool
            # quads between DVE quads so neither engine trails at head end
            order = []
            while pq or dq:
                if dq:
                    order.append(dq.pop(0))
                if pq:
                    order.append(pq.pop(0))
            for qi, q in enumerate(order):
                uq = work.tile([P, 4 * SL], F16, tag="uq", bufs=6,
                               name=f"uq{h}_{q}")
                for k in range(4):
                    jt = 4 * q + k
                    nc.vector.tensor_scalar(
                        uq[:, k * SL:(k + 1) * SL], g_bc[h][:],
                        rq[jt][:, h:h + 1], e1q[jt][:, h:h + 1],
                        A.max, A.mult)
                mq = work.tile([P, 4 * SL], F16, tag="mq", bufs=6,
                               name=f"mq{h}_{q}")
                if (h * 8 + q) in POOL_L1:
                    nc.gpsimd.tensor_mul(mq[:], uq[:], adjq_sb[q][:])
                else:
                    nc.vector.tensor_mul(mq[:], uq[:], adjq_sb[q][:])
                for k in range(4):
                    jt = 4 * q + k
                    nc.tensor.matmul(acc[:], lhst(h, jt),
                                     mq[:, k * SL:(k + 1) * SL],
                                     start=(qi == 0 and k == 0),
                                     stop=(qi == NQ - 1 and k == 3))
            # normalization + ELU, written into hT k-tiles (2 heads/tile)
            den_sb = work.tile([1, SL], F32, tag="den", bufs=2,
                               name=f"den{h}")
            nc.scalar.activation(den_sb[:], acc[D:D + 1, :], AF.Copy)
            den_bc = psum.tile([D, SL], F32, tag="bank", bufs=4,
                               name=f"dbc{h}")
            nc.tensor.matmul(den_bc[:], ones1[0:1, 0:D], den_sb[:],
                             start=True, stop=True)
            lnb = work.tile([D, SL], F32, tag="lnb", bufs=2, name=f"lnb{h}")
            nc.scalar.activation(lnb[:], den_bc[:], AF.Ln)
            recb = work.tile([D, SL], F32, tag="recb", bufs=2,
                             name=f"recb{h}")
            nc.scalar.activation(recb[:], lnb[:], AF.Exp, scale=-1.0)
            hsl = hT[h // 2][(h % 2) * D:(h % 2) * D + D, :]
            nc.vector.tensor_mul(hsl, acc[0:D, :], recb[:])
            # ELU in fp16: elu(x) = (x max 0 - 1) + exp(x min 0)
            tmin = work.tile([D, SL], F16, tag="tmin", bufs=2,
                             name=f"tmin{h}")
            nc.vector.tensor_scalar_min(tmin[:], hsl, 0.0)
            texp = work.tile([D, SL], F16, tag="texp", bufs=2,
                             name=f"texp{h}")
            nc.scalar.activation(texp[:], tmin[:], AF.Exp)
            tlin = work.tile([D, SL], F16, tag="tlin", bufs=2,
                             name=f"tlin{h}")
            nc.vector.tensor_scalar(tlin[:], hsl, 0.0, -1.0, A.max, A.add)
            nc.vector.tensor_add(hsl, texp[:], tlin[:])

        # ---- finish Wh2_ext (last head pair) and stage for the exchange --
        emit_wh2_part(3)
        cc_in = dram.tile([SL, 66], F16, tag="cc_in", name="cc_in")
        cc_full = dram.tile([N, 66], F16, tag="cc_full", addr_space="Shared",
                            name="cc_full")
        p2_sb = work.tile([P, 264], F16, tag="p2", name="p2")
        nc.scalar.activation(p2_sb[:], wh2acc[:], AF.Copy)
        nc.sync.dma_start(
            out=cc_in[:].rearrange("(k p) c -> p k c", k=4),
            in_=p2_sb[:].rearrange("p (k c) -> p k c", k=4))
        bc2_ps = psum.tile([P, SL], F32, tag="bank", bufs=4, name="gbc2")
        nc.tensor.matmul(bc2_ps[:], ones1[0:1, :], fr2acc[:], start=True,
                         stop=True)
        g2 = persist.tile([P, SL], F16, tag="g2", name="g2")
        nc.scalar.activation(g2[:], bc2_ps[:], AF.Exp, scale=0.8)

        nc.gpsimd.collective_compute(
            "AllGather", A.bypass, ins=[cc_in[:]], outs=[cc_full[:]],
            replica_groups=[list(range(N_CORES))])


        # ---- layer 2: two rearranged DMAs load the 8 prep groups --------
        cc_all = persist.tile([P, NQ * 264], F16, tag="cc_all", name="cc_all")
        HG = NQ // 2
        for gh in range(2):
            nc.sync.dma_start(
                out=cc_all[:, gh * HG * 264:(gh + 1) * HG * 264].rearrange(
                    "p (g k c) -> p g k c", g=HG, k=4),
                in_=cc_full[gh * N // 2:(gh + 1) * N // 2, :].rearrange(
                    "(g k p) c -> p g k c", g=HG, k=4))
        cc_gp = [cc_all[:, g * 264:(g + 1) * 264] for g in range(NQ)]
        e1q2, rq2 = [], []
        for g in range(NQ):
            gp = cc_gp[g]
            e1 = persist.tile([P, 4], F32, tag=f"e2_{g}", name=f"e2_{g}")
            nc.scalar.activation(e1[:], gp[:, 65:264:66], AF.Exp)
            r = persist.tile([P, 4], F32, tag=f"r2_{g}", name=f"r2_{g}")
            nc.scalar.activation(r[:], gp[:, 65:264:66], AF.Exp, scale=-0.8)
            nc.vector.memset(gp[:, 64:264:66], 1.0)
            e1q2.append(e1)
            rq2.append(r)

        # ---- layer-2 attention ------------------------------------------
        acc2 = psum.tile([D + 1, SL], F32, tag="acc", bufs=4, name="acc2")
        # Pool-assigned quads first so Pool's slow muls overlap the DVE quads
        l2_order = sorted(range(NQ), key=lambda q: q not in POOL_L2)
        for qi, q in enumerate(l2_order):
            uq = work.tile([P, 4 * SL], F16, tag="uq", bufs=6,
                           name=f"uq2_{q}")
            for k in range(4):
                nc.vector.tensor_scalar(
                    uq[:, k * SL:(k + 1) * SL], g2[:],
                    rq2[q][:, k:k + 1], e1q2[q][:, k:k + 1],
                    A.max, A.mult)
            mq = work.tile([P, 4 * SL], F16, tag="mq", bufs=6,
                           name=f"mq2_{q}")
            if q in POOL_L2:
                nc.gpsimd.tensor_mul(mq[:], uq[:], adjq_sb[q][:])
            else:
                nc.vector.tensor_mul(mq[:], uq[:], adjq_sb[q][:])
            for k in range(4):
                nc.tensor.matmul(acc2[:], cc_gp[q][:, k * 66:k * 66 + 65],
                                 mq[:, k * SL:(k + 1) * SL],
                                 start=(qi == 0 and k == 0),
                                 stop=(qi == NQ - 1 and k == 3))
        den2 = work.tile([1, SL], F32, tag="den", bufs=2, name="den2")
        nc.scalar.activation(den2[:], acc2[D:D + 1, :], AF.Copy)
        dbc2 = psum.tile([D, SL], F32, tag="bank", bufs=4, name="dbc2")
        nc.tensor.matmul(dbc2[:], ones1[0:1, 0:D], den2[:], start=True,
                         stop=True)
        ln2 = work.tile([D, SL], F32, tag="lnb", bufs=2, name="ln2")
        nc.scalar.activation(ln2[:], dbc2[:], AF.Ln)
        rec2 = work.tile([D, SL], F32, tag="recb", bufs=2, name="rec2")
        nc.scalar.activation(rec2[:], ln2[:], AF.Exp, scale=-1.0)
        o2 = persist.tile([D, SL], F32, tag="o2", name="o2")
        nc.vector.tensor_mul(o2[:], acc2[0:D, :], rec2[:])
        # final ELU in fp32
        t2min = work.tile([D, SL], F32, tag="t2min", name="t2min")
        nc.vector.tensor_scalar_min(t2min[:], o2[:], 0.0)
        t2exp = work.tile([D, SL], F32, tag="t2exp", name="t2exp")
        nc.scalar.activation(t2exp[:], t2min[:], AF.Exp)
        t2lin = work.tile([D, SL], F32, tag="t2lin", name="t2lin")
        nc.vector.tensor_scalar(t2lin[:], o2[:], 0.0, -1.0, A.max, A.add)
        fin = persist.tile([D, SL], F32, tag="fin", name="fin")
        nc.vector.tensor_add(fin[:], t2exp[:], t2lin[:])
        nc.sync.dma_start(out=outT[:], in_=fin[:])

    nc.compile()
    return nc


# ---------------------------------------------------------------------------
# host-side driver
# ---------------------------------------------------------------------------

def _prep_inputs(x, adj, W, a, Wo, ao):
    xT16 = np.ascontiguousarray(x.T.astype(np.float16))
    adjT16 = adj.T.astype(np.float16)
    wext = np.empty((F, H * 66), np.float32)
    for h in range(H):
        wext[:, h * 66:h * 66 + D] = W[h]
        wext[:, h * 66 + D] = W[h] @ a[h, :D]
        wext[:, h * 66 + D + 1] = W[h] @ a[h, D:]
    wext = wext.astype(np.float16)
    woext = np.concatenate(
        [Wo, (Wo @ ao[:D])[:, None], (Wo @ ao[D:])[:, None]],
        axis=1).astype(np.float16)

    in_maps = []
    for c in range(N_CORES):
        sl = slice(c * SL, (c + 1) * SL)
        adjq = np.empty((NQ * P, 4 * SL), np.float16)
        for q in range(NQ):
            for k in range(4):
                jt = 4 * q + k
                adjq[q * P:(q + 1) * P, k * SL:(k + 1) * SL] = \
                    adjT16[jt * P:(jt + 1) * P, sl]
        selD = np.zeros((H, H * P), np.float16)
        for h in range(H):
            selD[h, h * P:(h + 1) * P] = 1.0
        in_maps.append({
            "xT": xT16,
            "xS": np.ascontiguousarray(xT16[:, sl]),
            "adjQ": adjq,
            "Wext": wext,
            "Woext": woext,
            "selD": selD,
        })
    return in_maps


def kernel(x, adj, W, a, Wo, ao, cfg):
    x = np.asarray(x, np.float32)
    adj = np.asarray(adj, np.float32)
    W = np.asarray(W, np.float32)
    a = np.asarray(a, np.float32)
    Wo = np.asarray(Wo, np.float32)
    ao = np.asarray(ao, np.float32)

    in_maps = _prep_inputs(x, adj, W, a, Wo, ao)
    if _CACHED.get("nc") is None:
        _CACHED["nc"] = build_kernel()
    res = run_bass_kernel_spmd(_CACHED["nc"], in_maps,
                               core_ids=list(range(N_CORES)))
    out = np.empty((N, D), np.float32)
    for c in range(N_CORES):
        out[c * SL:(c + 1) * SL, :] = res.results[c]["outT"].T
    return out


if __name__ == "__main__":
    import reference as ref_mod
    inputs = {k: np.asarray(v) for k, v in ref_mod.setup_inputs().items()}
    expected = np.asarray(ref_mod.reference(**ref_mod.setup_inputs()))
    got = kernel(**inputs)
    err = np.abs(got - expected).max() / np.abs(expected).max()
    print("rel err:", err)



# revision 3
# speedup vs baseline: 1.1275x; 1.1275x over previous
"""GAT (2-layer graph attention network) Trainium2 Bass kernel, 8-core SPMD.

Sharding: core c owns output rows i in [c*512, (c+1)*512) for BOTH layers and
computes ALL 8 heads of layer 1 for those rows (column/row-parallel instead of
head-parallel). Wins vs head-parallel: the adjacency stripe adjT[:, slice] is
loaded once (4MB fp16) and reused by all 8 heads AND layer 2 (8x less HBM
traffic than one full adj per core), and the inter-layer exchange collapses to
a single AllGather of h @ Wo_ext [4096, 66] fp16 (no ReduceScatter at all,
since each core holds complete h rows).

Math: with s_ij = f_src[i] + f_dst[j], exp(lrelu(s)) equals, up to a per-i
factor that cancels in softmax, max(g[i], r[j]) * e1[j] with
g = exp(0.8*f_src), r = exp(-0.8*f_dst), e1 = exp(f_dst - C). So the masked
unnormalized score matrix is M[j,i] = adj[j,i] * u[j,i] against a RAW lhsT
[Wh | 1], where u = (g max r) * e1 is ONE dual-op DVE tensor_scalar (4x fp16
mode). Each quad's mask multiply is split by free range between Pool
([0:SPOOL], TT-mult is the only elementwise op Pool's silicon accepts) and
DVE ([SPOOL:2048], 2x fp16 mode) so both engines stay balanced every quad
instead of whole quads ping-ponging between engines.

kernel(**inputs) takes full unsharded inputs, returns the full output.
"""

from contextlib import ExitStack

import numpy as np

import concourse.mybir as mybir
import concourse.tile as tile
from concourse import bacc
from concourse.bass_utils import run_bass_kernel_spmd

# Steer every activation to the one ACT table set covering all functions this
# kernel uses (Exp, Identity, Ln, Copy, Relu) so no mid-kernel table reloads.
_orig_get_tables = bacc.get_activation_tables


def _pinned_tables(arch):
    tabs = _orig_get_tables(arch)
    if "natural_log_exp_and_others" in tabs:
        return {name: (funcs if name == "natural_log_exp_and_others" else set())
                for name, funcs in tabs.items()}
    return tabs


bacc.get_activation_tables = _pinned_tables

N = 4096
F = 512
D = 64          # per-head hidden == n classes
H = 8
P = 128
NT = N // P     # 32 j tiles
SL = 512        # i columns per core
NKF = F // P    # 4 contraction tiles for x @ W
NQ = 8          # j quads (4 j-tiles each)
C_DST = 7.0     # layer-1 exponent shift: keeps u = max(g,r)*e1 under fp16 max
N_CORES = 8
SPOOL = 768     # free-range split of each quad's mask op: Pool [0:SPOOL],
                # DVE [SPOOL:4*SL]. Pool runs TT-mult at 0.42 GPSIMD
                # efficiency (the only elementwise ALU op the silicon
                # accepts on Pool), DVE at the 2x fp16 mode, so Pool gets
                # the smaller share.

F32 = mybir.dt.float32
F16 = mybir.dt.float16
A = mybir.AluOpType
AF = mybir.ActivationFunctionType

_CACHED = {}


def build_kernel():
    nc = bacc.Bacc("TRN2", num_devices=N_CORES)

    xT = nc.dram_tensor("xT", [F, N], F16, kind="ExternalInput")
    xS = nc.dram_tensor("xS", [F, SL], F16, kind="ExternalInput")
    adjQ = nc.dram_tensor("adjQ", [NQ * P, 4 * SL], F16, kind="ExternalInput")
    Wext = nc.dram_tensor("Wext", [F, H * 66], F16, kind="ExternalInput")
    selD = nc.dram_tensor("selD", [H, H * P], F16, kind="ExternalInput")
    Woext = nc.dram_tensor("Woext", [F, 66], F16, kind="ExternalInput")
    outT = nc.dram_tensor("outT", [D, SL], F32, kind="ExternalOutput")

    with ExitStack() as ctx:
        tc = ctx.enter_context(tile.TileContext(nc))
        psum = ctx.enter_context(tc.tile_pool(name="psum", bufs=1, space="PSUM"))
        persist = ctx.enter_context(tc.tile_pool(name="persist", bufs=1))
        work = ctx.enter_context(tc.tile_pool(name="work", bufs=1))
        dram = ctx.enter_context(tc.tile_pool(name="dram", bufs=1, space="DRAM"))

        ones1 = persist.tile([1, P], F32, tag="ones1")
        nc.vector.memset(ones1[:], 1.0)
        bias_c = persist.tile([P, 1], F32, tag="bias_c")
        nc.vector.memset(bias_c[:], -C_DST)

        # ---- input DMAs (sel first: tiny, and emit_g head-of-line blocks
        # the PE queue on it). Few, large DMAs: each dispatch serializes
        # ~0.6-1.2us on SP.SEQ/HWDGE, so k-tiles are packed side by side in
        # one SBUF tile per tensor and sliced at use. ---------------------
        sel = persist.tile([H, H * P], F16, tag="sel", name="sel")
        nc.sync.dma_start(out=sel[:], in_=selD[:])
        xs_all = persist.tile([P, NKF * SL], F16, tag="xs", name="xs")
        nc.sync.dma_start(
            out=xs_all[:].rearrange("p (k c) -> p k c", k=NKF),
            in_=xS[:].rearrange("(k p) c -> p k c", k=NKF))
        wext_all = persist.tile([P, NKF * 528], F16, tag="we", name="we")
        nc.sync.dma_start(
            out=wext_all[:].rearrange("p (k c) -> p k c", k=NKF),
            in_=Wext[:].rearrange("(k p) c -> p k c", k=NKF))

        def xsl(kf):
            return xs_all[:, kf * SL:(kf + 1) * SL]

        def wsl(kf, lo, hi, step=1):
            return wext_all[:, kf * 528 + lo:kf * 528 + hi:step]

        xt_sb = [persist.tile([P, N], F16, tag=f"xt{kf}", name=f"xt{kf}")
                 for kf in range(NKF)]
        # adjacency mask: +inf where edge, 0 where none (min-mask trick).
        # 4 tiles of 2 quads each; quad q = adj2_sb[q//2][:, (q%2)*2048:...].
        adj2_sb = [persist.tile([P, 8 * SL], F16, tag=f"adj{qq}",
                                name=f"adj{qq}") for qq in range(NQ // 2)]

        def adj_q(q):
            return adj2_sb[q // 2][:, (q % 2) * 4 * SL:((q % 2) + 1) * 4 * SL]

        # j-tiles 0-7 first (first attention quads), then 8-15, then rest
        for kf in range(NKF):
            nc.sync.dma_start(out=xt_sb[kf][:, 0:1024],
                              in_=xT[kf * P:(kf + 1) * P, 0:1024])
        nc.sync.dma_start(
            out=adj2_sb[0][:].rearrange("p (two c) -> p two c", two=2),
            in_=adjQ[0:2 * P, :].rearrange("(two p) c -> p two c", two=2))
        for kf in range(NKF):
            nc.sync.dma_start(out=xt_sb[kf][:, 1024:2048],
                              in_=xT[kf * P:(kf + 1) * P, 1024:2048])
        nc.sync.dma_start(
            out=adj2_sb[1][:].rearrange("p (two c) -> p two c", two=2),
            in_=adjQ[2 * P:4 * P, :].rearrange("(two p) c -> p two c", two=2))
        for kf in range(NKF):
            nc.sync.dma_start(out=xt_sb[kf][:, 2048:4096],
                              in_=xT[kf * P:(kf + 1) * P, 2048:4096])
        for qq in (2, 3):
            nc.sync.dma_start(
                out=adj2_sb[qq][:].rearrange("p (two c) -> p two c", two=2),
                in_=adjQ[2 * qq * P:2 * (qq + 1) * P, :].rearrange(
                    "(two p) c -> p two c", two=2))
        woext_all = persist.tile([P, NKF * 66], F16, tag="wo", name="wo")
        nc.sync.dma_start(
            out=woext_all[:].rearrange("p (k c) -> p k c", k=NKF),
            in_=Woext[:].rearrange("(k p) c -> p k c", k=NKF))

        # ---- f_src rows for all 8 heads in ONE matmul chain -------------
        # lhsT = strided f_src weight columns [128, 8] -> fr_ps [8, 512]
        fr_ps = psum.tile([H, SL], F32, tag="bank", bufs=4, name="fr")
        for kf in range(NKF):
            nc.tensor.matmul(fr_ps[:], wsl(kf, 64, 528, 66), xsl(kf),
                             start=(kf == 0), stop=(kf == NKF - 1))
        fsr = persist.tile([H, SL], F16, tag="fsr", name="fsr")
        nc.scalar.activation(fsr[:], fr_ps[:], AF.Copy)
        # sel: one-hot selector tiles, sel[k, h*128+p] = (k == h), so a K=8
        # matmul against the full fsr broadcasts row h down 128 partitions
        # without a partition-offset rhs (which BIR rejects). Host-provided.
        g_bc = [persist.tile([P, SL], F16, tag=f"g{h}", name=f"g{h}")
                for h in range(H)]

        def emit_g(h):
            # broadcast row h of fsr down 128 partitions, then exp(0.8 x)
            bc_ps = psum.tile([P, SL], F32, tag="bank", bufs=4, name=f"gb{h}")
            nc.tensor.matmul(bc_ps[:], sel[:, h * P:(h + 1) * P], fsr[:],
                             start=True, stop=True)
            nc.scalar.activation(g_bc[h][:], bc_ps[:], AF.Exp, scale=0.8)

        # ---- stage prep: Wh_ext tiles (4 heads wide), e1/r, ones col ----
        # stage[t][qd] = fp16 [Wh_h|f_src_h|f_dst_h for h in 4qd..4qd+3],
        # then col 64 of each head slot is overwritten with 1.0 (the
        # denominator column); f_dst (col 65) stays for e1/r extraction.
        stage = [persist.tile([P, 528], F16, tag=f"st{t}", name=f"st{t}")
                 for t in range(NT)]
        e1q = [persist.tile([P, H], F32, tag=f"e1_{t}", name=f"e1_{t}")
               for t in range(NT)]
        rq = [persist.tile([P, H], F32, tag=f"r_{t}", name=f"r_{t}")
              for t in range(NT)]

        def emit_stage_half(t, qd):
            """Wh_ext matmuls + fp16 staging + e1/r + ones col for heads
            [4qd, 4qd+4) at j-tile t."""
            st = stage[t]
            wh_ps = psum.tile([P, 264], F32, tag="bank", bufs=4,
                              name=f"wh{t}_{qd}")
            for kf in range(NKF):
                nc.tensor.matmul(
                    wh_ps[:], xt_sb[kf][:, t * P:(t + 1) * P],
                    wsl(kf, qd * 264, (qd + 1) * 264),
                    start=(kf == 0), stop=(kf == NKF - 1))
            lo = qd * 264
            nc.scalar.activation(st[:, lo:lo + 264], wh_ps[:], AF.Copy)
            nc.scalar.activation(e1q[t][:, 4 * qd:4 * qd + 4],
                                 st[:, lo + 65:lo + 264:66], AF.Exp,
                                 bias=bias_c[:])
            nc.scalar.activation(rq[t][:, 4 * qd:4 * qd + 4],
                                 st[:, lo + 65:lo + 264:66], AF.Exp,
                                 scale=-0.8)
            nc.vector.memset(st[:, lo + 64:lo + 264:66], 1.0)

        # heads 0-3 stages first (with g broadcasts interleaved); heads 4-7
        # stages are emitted between the first heads' attention chains below,
        # filling PE gaps and avoiding a long serial prep phase.
        for t in range(NT):
            if t < H:
                emit_g(t)
            emit_stage_half(t, 0)

        def lhst(h, jt):
            return stage[jt][:, h * 66:h * 66 + 65]

        # ---- layer-1 attention: h outer, quad q inner -------------------
        hT = [persist.tile([P, SL], F16, tag=f"hT{kt}", name=f"hT{kt}")
              for kt in range(NKF)]
        # Wh2 = h @ Wo_ext and the f_src2 row, accumulated one kt (head pair)
        # at a time right after that pair's hT half is written, so only the
        # last kt's small matmuls sit on the pre-AllGather critical path
        wh2acc = persist.tile([P, 264], F32, tag="wh2acc", name="wh2acc")
        fr2acc = persist.tile([1, SL], F32, tag="fr2acc", name="fr2acc")

        def emit_wh2_part(kt):
            o2_ps = psum.tile([P, 264], F32, tag="bank", bufs=4,
                              name=f"o2p{kt}")
            for sub in range(4):
                nc.tensor.matmul(o2_ps[:, sub * 66:(sub + 1) * 66],
                                 hT[kt][:, sub * P:(sub + 1) * P],
                                 woext_all[:, kt * 66:(kt + 1) * 66],
                                 start=True, stop=True)
            fr2_ps = psum.tile([1, SL], F32, tag="bank", bufs=4,
                               name=f"fr2p{kt}")
            nc.tensor.matmul(fr2_ps[:], woext_all[:, kt * 66 + 64:kt * 66 + 65],
                             hT[kt][:], start=True, stop=True)
            if kt == 0:
                nc.scalar.activation(wh2acc[:], o2_ps[:], AF.Copy)
                nc.scalar.activation(fr2acc[:], fr2_ps[:], AF.Copy)
            else:
                nc.vector.tensor_add(wh2acc[:], wh2acc[:], o2_ps[:])
                nc.vector.tensor_add(fr2acc[:], fr2acc[:], fr2_ps[:])

        def emit_masked_quad(uq_name, mq_name, g_t, rq_t, e1q_t, adj_ap,
                             sub=None):
            """TSPs + split Pool/DVE min-mask for one quad. rq_t/e1q_t are
            lists of ([P,1] AP) per k chunk. Returns the masked mq tile."""
            uq = work.tile([P, 4 * SL], F16, tag="uq", bufs=6, name=uq_name)
            mq = work.tile([P, 4 * SL], F16, tag="mq", bufs=6, name=mq_name)
            for k in range(2):
                nc.vector.tensor_scalar(
                    uq[:, k * SL:(k + 1) * SL], g_t, rq_t[k], e1q_t[k],
                    A.max, A.mult)
            nc.gpsimd.tensor_tensor(mq[:, 0:SPOOL], uq[:, 0:SPOOL],
                                    adj_ap[:, 0:SPOOL], A.mult)
            for k in range(2, 4):
                nc.vector.tensor_scalar(
                    uq[:, k * SL:(k + 1) * SL], g_t, rq_t[k], e1q_t[k],
                    A.max, A.mult)
            nc.vector.tensor_tensor(mq[:, SPOOL:4 * SL], uq[:, SPOOL:4 * SL],
                                    adj_ap[:, SPOOL:4 * SL], A.mult)
            return mq

        MM_ORDER = (2, 3, 0, 1)  # DVE-half chunks first: they finish earlier

        for h in range(H):
            if h < 4:
                for t in range(h * 8, h * 8 + 8):
                    emit_stage_half(t, 1)
            if h % 2 == 1 and h > 1:
                emit_wh2_part(h // 2 - 1)
            acc = psum.tile([D + 1, SL], F32, tag="acc", bufs=4,
                            name=f"acc{h}")
            for q in range(NQ):
                jts = [4 * q + k for k in range(4)]
                mq = emit_masked_quad(
                    f"uq{h}_{q}", f"mq{h}_{q}", g_bc[h][:],
                    [rq[jt][:, h:h + 1] for jt in jts],
                    [e1q[jt][:, h:h + 1] for jt in jts], adj_q(q))
                for ki, k in enumerate(MM_ORDER):
                    nc.tensor.matmul(acc[:], lhst(h, jts[k]),
                                     mq[:, k * SL:(k + 1) * SL],
                                     start=(q == 0 and ki == 0),
                                     stop=(q == NQ - 1 and ki == 3))
            # normalization + ELU, written into hT k-tiles (2 heads/tile)
            den_sb = work.tile([1, SL], F32, tag="den", bufs=2,
                               name=f"den{h}")
            nc.scalar.activation(den_sb[:], acc[D:D + 1, :], AF.Copy)
            den_bc = psum.tile([D, SL], F32, tag="bank", bufs=4,
                               name=f"dbc{h}")
            nc.tensor.matmul(den_bc[:], ones1[0:1, 0:D], den_sb[:],
                             start=True, stop=True)
            lnb = work.tile([D, SL], F32, tag="lnb", bufs=2, name=f"lnb{h}")
            nc.scalar.activation(lnb[:], den_bc[:], AF.Ln)
            recb = work.tile([D, SL], F32, tag="recb", bufs=2,
                             name=f"recb{h}")
            nc.scalar.activation(recb[:], lnb[:], AF.Exp, scale=-1.0)
            hsl = hT[h // 2][(h % 2) * D:(h % 2) * D + D, :]
            nc.vector.tensor_mul(hsl, acc[0:D, :], recb[:])
            # ELU in fp16: elu(x) = (x max 0 - 1) + exp(-relu(-x));
            # relu(-x) runs on ACT (scale=-1) to keep DVE light.
            a1 = work.tile([D, SL], F16, tag="a1", bufs=2, name=f"a1{h}")
            nc.scalar.activation(a1[:], hsl, AF.Relu, scale=-1.0)
            texp = work.tile([D, SL], F16, tag="texp", bufs=2,
                             name=f"texp{h}")
            nc.scalar.activation(texp[:], a1[:], AF.Exp, scale=-1.0)
            tlin = work.tile([D, SL], F16, tag="tlin", bufs=2,
                             name=f"tlin{h}")
            nc.vector.tensor_scalar(tlin[:], hsl, 0.0, -1.0, A.max, A.add)
            nc.vector.tensor_add(hsl, texp[:], tlin[:])

        # ---- finish Wh2_ext (last head pair) and stage for the exchange --
        emit_wh2_part(3)
        cc_in = dram.tile([SL, 66], F16, tag="cc_in", name="cc_in")
        cc_full = dram.tile([N, 66], F16, tag="cc_full", addr_space="Shared",
                            name="cc_full")
        p2_sb = work.tile([P, 264], F16, tag="p2", name="p2")
        nc.scalar.activation(p2_sb[:], wh2acc[:], AF.Copy)
        nc.sync.dma_start(
            out=cc_in[:].rearrange("(k p) c -> p k c", k=4),
            in_=p2_sb[:].rearrange("p (k c) -> p k c", k=4))
        bc2_ps = psum.tile([P, SL], F32, tag="bank", bufs=4, name="gbc2")
        nc.tensor.matmul(bc2_ps[:], ones1[0:1, :], fr2acc[:], start=True,
                         stop=True)
        g2 = persist.tile([P, SL], F16, tag="g2", name="g2")
        nc.scalar.activation(g2[:], bc2_ps[:], AF.Exp, scale=0.8)

        nc.gpsimd.collective_compute(
            "AllGather", A.bypass, ins=[cc_in[:]], outs=[cc_full[:]],
            replica_groups=[list(range(N_CORES))])

        # ---- layer 2: two rearranged DMAs load the 8 prep groups --------
        cc_all = persist.tile([P, NQ * 264], F16, tag="cc_all", name="cc_all")
        HG = NQ // 2
        for gh in range(2):
            nc.sync.dma_start(
                out=cc_all[:, gh * HG * 264:(gh + 1) * HG * 264].rearrange(
                    "p (g k c) -> p g k c", g=HG, k=4),
                in_=cc_full[gh * N // 2:(gh + 1) * N // 2, :].rearrange(
                    "(g k p) c -> p g k c", g=HG, k=4))
        cc_gp = [cc_all[:, g * 264:(g + 1) * 264] for g in range(NQ)]
        e1q2, rq2 = [], []
        for g in range(NQ):
            gp = cc_gp[g]
            e1 = persist.tile([P, 4], F32, tag=f"e2_{g}", name=f"e2_{g}")
            nc.scalar.activation(e1[:], gp[:, 65:264:66], AF.Exp)
            r = persist.tile([P, 4], F32, tag=f"r2_{g}", name=f"r2_{g}")
            nc.scalar.activation(r[:], gp[:, 65:264:66], AF.Exp, scale=-0.8)
            nc.vector.memset(gp[:, 64:264:66], 1.0)
            e1q2.append(e1)
            rq2.append(r)

        # ---- layer-2 attention ------------------------------------------
        acc2 = psum.tile([D + 1, SL], F32, tag="acc", bufs=4, name="acc2")
        for q in range(NQ):
            mq = emit_masked_quad(
                f"uq2_{q}", f"mq2_{q}", g2[:],
                [rq2[q][:, k:k + 1] for k in range(4)],
                [e1q2[q][:, k:k + 1] for k in range(4)], adj_q(q))
            for ki, k in enumerate(MM_ORDER):
                nc.tensor.matmul(acc2[:], cc_gp[q][:, k * 66:k * 66 + 65],
                                 mq[:, k * SL:(k + 1) * SL],
                                 start=(q == 0 and ki == 0),
                                 stop=(q == NQ - 1 and ki == 3))
        den2 = work.tile([1, SL], F32, tag="den", bufs=2, name="den2")
        nc.scalar.activation(den2[:], acc2[D:D + 1, :], AF.Copy)
        dbc2 = psum.tile([D, SL], F32, tag="bank", bufs=4, name="dbc2")
        nc.tensor.matmul(dbc2[:], ones1[0:1, 0:D], den2[:], start=True,
                         stop=True)
        ln2 = work.tile([D, SL], F32, tag="lnb", bufs=2, name="ln2")
        nc.scalar.activation(ln2[:], dbc2[:], AF.Ln)
        rec2 = work.tile([D, SL], F32, tag="recb", bufs=2, name="rec2")
        nc.scalar.activation(rec2[:], ln2[:], AF.Exp, scale=-1.0)
        o2 = persist.tile([D, SL], F32, tag="o2", name="o2")
        nc.vector.tensor_mul(o2[:], acc2[0:D, :], rec2[:])
        # final ELU in fp32 (same relu(-x) decomposition, ACT-heavy)
        a2 = work.tile([D, SL], F32, tag="a2", name="a2")
        nc.scalar.activation(a2[:], o2[:], AF.Relu, scale=-1.0)
        t2exp = work.tile([D, SL], F32, tag="t2exp", name="t2exp")
        nc.scalar.activation(t2exp[:], a2[:], AF.Exp, scale=-1.0)
        t2lin = work.tile([D, SL], F32, tag="t2lin", name="t2lin")
        nc.vector.tensor_scalar(t2lin[:], o2[:], 0.0, -1.0, A.max, A.add)
        fin = persist.tile([D, SL], F32, tag="fin", name="fin")
        nc.vector.tensor_add(fin[:], t2exp[:], t2lin[:])
        nc.sync.dma_start(out=outT[:], in_=fin[:])

    nc.compile()
    return nc


# ---------------------------------------------------------------------------
# host-side driver
# ---------------------------------------------------------------------------

def _prep_inputs(x, adj, W, a, Wo, ao):
    xT16 = np.ascontiguousarray(x.T.astype(np.float16))
    adjT16 = adj.T.astype(np.float16)
    wext = np.empty((F, H * 66), np.float32)
    for h in range(H):
        wext[:, h * 66:h * 66 + D] = W[h]
        wext[:, h * 66 + D] = W[h] @ a[h, :D]
        wext[:, h * 66 + D + 1] = W[h] @ a[h, D:]
    wext = wext.astype(np.float16)
    woext = np.concatenate(
        [Wo, (Wo @ ao[:D])[:, None], (Wo @ ao[D:])[:, None]],
        axis=1).astype(np.float16)

    in_maps = []
    for c in range(N_CORES):
        sl = slice(c * SL, (c + 1) * SL)
        adjq = np.empty((NQ * P, 4 * SL), np.float16)
        for q in range(NQ):
            for k in range(4):
                jt = 4 * q + k
                adjq[q * P:(q + 1) * P, k * SL:(k + 1) * SL] = \
                    adjT16[jt * P:(jt + 1) * P, sl]
        selD = np.zeros((H, H * P), np.float16)
        for h in range(H):
            selD[h, h * P:(h + 1) * P] = 1.0
        in_maps.append({
            "xT": xT16,
            "xS": np.ascontiguousarray(xT16[:, sl]),
            "adjQ": adjq,
            "Wext": wext,
            "Woext": woext,
            "selD": selD,
        })
    return in_maps


def kernel(x, adj, W, a, Wo, ao, cfg):
    x = np.asarray(x, np.float32)
    adj = np.asarray(adj, np.float32)
    W = np.asarray(W, np.float32)
    a = np.asarray(a, np.float32)
    Wo = np.asarray(Wo, np.float32)
    ao = np.asarray(ao, np.float32)

    in_maps = _prep_inputs(x, adj, W, a, Wo, ao)
    if _CACHED.get("nc") is None:
        _CACHED["nc"] = build_kernel()
    res = run_bass_kernel_spmd(_CACHED["nc"], in_maps,
                               core_ids=list(range(N_CORES)))
    out = np.empty((N, D), np.float32)
    for c in range(N_CORES):
        out[c * SL:(c + 1) * SL, :] = res.results[c]["outT"].T
    return out


if __name__ == "__main__":
    import reference as ref_mod
    inputs = {k: np.asarray(v) for k, v in ref_mod.setup_inputs().items()}
    expected = np.asarray(ref_mod.reference(**ref_mod.setup_inputs()))
    got = kernel(**inputs)
    err = np.abs(got - expected).max() / np.abs(expected).max()
    print("rel err:", err)


# revision 6
# speedup vs baseline: 1.1373x; 1.0087x over previous
"""GAT (2-layer graph attention network) Trainium2 Bass kernel, 8-core SPMD.

Sharding: core c owns output rows i in [c*512, (c+1)*512) for BOTH layers and
computes ALL 8 heads of layer 1 for those rows (column/row-parallel instead of
head-parallel). Wins vs head-parallel: the adjacency stripe adjT[:, slice] is
loaded once (4MB fp16) and reused by all 8 heads AND layer 2 (8x less HBM
traffic than one full adj per core), and the inter-layer exchange collapses to
a single AllGather of h @ Wo_ext [4096, 66] fp16 (no ReduceScatter at all,
since each core holds complete h rows).

Math: with s_ij = f_src[i] + f_dst[j], exp(lrelu(s)) equals, up to a per-i
factor that cancels in softmax, max(g[i], r[j]) * e1[j] with
g = exp(0.8*f_src), r = exp(-0.8*f_dst), e1 = exp(f_dst - C). So the masked
unnormalized score matrix is M[j,i] = adj[j,i] * u[j,i] against a RAW lhsT
[Wh | 1], where u = (g max r) * e1 is ONE dual-op DVE tensor_scalar (4x fp16
mode). Each quad's mask multiply is split by free range between Pool
([0:SPOOL], TT-mult is the only elementwise op Pool's silicon accepts) and
DVE ([SPOOL:2048], 2x fp16 mode) so both engines stay balanced every quad
instead of whole quads ping-ponging between engines.

kernel(**inputs) takes full unsharded inputs, returns the full output.
"""

from contextlib import ExitStack

import numpy as np

import concourse.mybir as mybir
import concourse.tile as tile
from concourse import bacc
from concourse.bass_utils import run_bass_kernel_spmd

# Steer every activation to the one ACT table set covering all functions this
# kernel uses (Exp, Identity, Ln, Copy, Relu) so no mid-kernel table reloads.
_orig_get_tables = bacc.get_activation_tables


def _pinned_tables(arch):
    tabs = _orig_get_tables(arch)
    if "natural_log_exp_and_others" in tabs:
        return {name: (funcs if name == "natural_log_exp_and_others" else set())
                for name, funcs in tabs.items()}
    return tabs


bacc.get_activation_tables = _pinned_tables

N = 4096
F = 512
D = 64          # per-head hidden == n classes
H = 8
P = 128
NT = N // P     # 32 j tiles
SL = 512        # i columns per core
NKF = F // P    # 4 contraction tiles for x @ W
NQ = 8          # j quads (4 j-tiles each)
C_DST = 7.0     # layer-1 exponent shift: keeps u = max(g,r)*e1 under fp16 max
N_CORES = 8
SPOOL = 832     # free-range split of each quad's mask op: Pool [0:SPOOL],
                # DVE [SPOOL:4*SL]. Pool runs TT-mult at 0.42 GPSIMD
                # efficiency (the only elementwise ALU op the silicon
                # accepts on Pool), DVE at the 2x fp16 mode, so Pool gets
                # the smaller share.

F32 = mybir.dt.float32
F16 = mybir.dt.float16
F8 = mybir.dt.float8e4
A = mybir.AluOpType
AF = mybir.ActivationFunctionType

_CACHED = {}


def build_kernel():
    nc = bacc.Bacc("TRN2", num_devices=N_CORES)

    xT = nc.dram_tensor("xT", [F, N], F16, kind="ExternalInput")
    xS = nc.dram_tensor("xS", [F, SL], F16, kind="ExternalInput")
    adjQ = nc.dram_tensor("adjQ", [NQ * P, 4 * SL], F16, kind="ExternalInput")
    Wext = nc.dram_tensor("Wext", [F, H * 66], F16, kind="ExternalInput")
    selD = nc.dram_tensor("selD", [H, H * P], F16, kind="ExternalInput")
    Woext = nc.dram_tensor("Woext", [F, 66], F16, kind="ExternalInput")
    outT = nc.dram_tensor("outT", [D, SL], F32, kind="ExternalOutput")

    with ExitStack() as ctx:
        tc = ctx.enter_context(tile.TileContext(nc))
        psum = ctx.enter_context(tc.tile_pool(name="psum", bufs=1, space="PSUM"))
        persist = ctx.enter_context(tc.tile_pool(name="persist", bufs=1))
        work = ctx.enter_context(tc.tile_pool(name="work", bufs=1))
        dram = ctx.enter_context(tc.tile_pool(name="dram", bufs=1, space="DRAM"))

        ones1 = persist.tile([1, P], F32, tag="ones1")
        nc.vector.memset(ones1[:], 1.0)
        bias_c = persist.tile([P, 1], F32, tag="bias_c")
        nc.vector.memset(bias_c[:], -C_DST)

        # ---- input DMAs (sel first: tiny, and emit_g head-of-line blocks
        # the PE queue on it). Few, large DMAs: each dispatch serializes
        # ~0.6-1.2us on SP.SEQ/HWDGE, so k-tiles are packed side by side in
        # one SBUF tile per tensor and sliced at use. ---------------------
        sel = persist.tile([H, H * P], F16, tag="sel", name="sel")
        nc.sync.dma_start(out=sel[:], in_=selD[:])
        xs_all = persist.tile([P, NKF * SL], F16, tag="xs", name="xs")
        nc.sync.dma_start(
            out=xs_all[:].rearrange("p (k c) -> p k c", k=NKF),
            in_=xS[:].rearrange("(k p) c -> p k c", k=NKF))
        wext_all = persist.tile([P, NKF * 528], F16, tag="we", name="we")
        nc.sync.dma_start(
            out=wext_all[:].rearrange("p (k c) -> p k c", k=NKF),
            in_=Wext[:].rearrange("(k p) c -> p k c", k=NKF))

        def xsl(kf):
            return xs_all[:, kf * SL:(kf + 1) * SL]

        def wsl(kf, lo, hi, step=1):
            return wext_all[:, kf * 528 + lo:kf * 528 + hi:step]

        xt_sb = [persist.tile([P, N], F16, tag=f"xt{kf}", name=f"xt{kf}")
                 for kf in range(NKF)]
        # adjacency mask: +inf where edge, 0 where none (min-mask trick).
        # 4 tiles of 2 quads each; quad q = adj2_sb[q//2][:, (q%2)*2048:...].
        adj2_sb = [persist.tile([P, 8 * SL], F16, tag=f"adj{qq}",
                                name=f"adj{qq}") for qq in range(NQ // 2)]

        def adj_q(q):
            return adj2_sb[q // 2][:, (q % 2) * 4 * SL:((q % 2) + 1) * 4 * SL]

        # j-tiles 0-7 first (first attention quads), then 8-15, then rest
        for kf in range(NKF):
            nc.sync.dma_start(out=xt_sb[kf][:, 0:1024],
                              in_=xT[kf * P:(kf + 1) * P, 0:1024])
        nc.sync.dma_start(
            out=adj2_sb[0][:].rearrange("p (two c) -> p two c", two=2),
            in_=adjQ[0:2 * P, :].rearrange("(two p) c -> p two c", two=2))
        for kf in range(NKF):
            nc.sync.dma_start(out=xt_sb[kf][:, 1024:2048],
                              in_=xT[kf * P:(kf + 1) * P, 1024:2048])
        nc.sync.dma_start(
            out=adj2_sb[1][:].rearrange("p (two c) -> p two c", two=2),
            in_=adjQ[2 * P:4 * P, :].rearrange("(two p) c -> p two c", two=2))
        for kf in range(NKF):
            nc.sync.dma_start(out=xt_sb[kf][:, 2048:4096],
                              in_=xT[kf * P:(kf + 1) * P, 2048:4096])
        for qq in (2, 3):
            nc.sync.dma_start(
                out=adj2_sb[qq][:].rearrange("p (two c) -> p two c", two=2),
                in_=adjQ[2 * qq * P:2 * (qq + 1) * P, :].rearrange(
                    "(two p) c -> p two c", two=2))
        woext_all = persist.tile([P, NKF * 66], F16, tag="wo", name="wo")
        nc.sync.dma_start(
            out=woext_all[:].rearrange("p (k c) -> p k c", k=NKF),
            in_=Woext[:].rearrange("(k p) c -> p k c", k=NKF))

        # ---- f_src rows for all 8 heads in ONE matmul chain -------------
        # lhsT = strided f_src weight columns [128, 8] -> fr_ps [8, 512]
        fr_ps = psum.tile([H, SL], F32, tag="bank", bufs=4, name="fr")
        for kf in range(NKF):
            nc.tensor.matmul(fr_ps[:], wsl(kf, 64, 528, 66), xsl(kf),
                             start=(kf == 0), stop=(kf == NKF - 1))
        fsr = persist.tile([H, SL], F16, tag="fsr", name="fsr")
        nc.scalar.activation(fsr[:], fr_ps[:], AF.Copy)
        # sel: one-hot selector tiles, sel[k, h*128+p] = (k == h), so a K=8
        # matmul against the full fsr broadcasts row h down 128 partitions
        # without a partition-offset rhs (which BIR rejects). Host-provided.
        g_bc = [persist.tile([P, SL], F16, tag=f"g{h}", name=f"g{h}")
                for h in range(H)]

        def emit_g(h):
            # broadcast row h of fsr down 128 partitions, then exp(0.8 x)
            bc_ps = psum.tile([P, SL], F32, tag="bank", bufs=4, name=f"gb{h}")
            nc.tensor.matmul(bc_ps[:], sel[:, h * P:(h + 1) * P], fsr[:],
                             start=True, stop=True)
            nc.scalar.activation(g_bc[h][:], bc_ps[:], AF.Exp, scale=0.8)

        # ---- stage prep: Wh_ext tiles (4 heads wide), e1/r, ones col ----
        # stage[t][qd] = fp16 [Wh_h|f_src_h|f_dst_h for h in 4qd..4qd+3],
        # then col 64 of each head slot is overwritten with 1.0 (the
        # denominator column); f_dst (col 65) stays for e1/r extraction.
        stage = [persist.tile([P, 528], F16, tag=f"st{t}", name=f"st{t}")
                 for t in range(NT)]
        e1q = [persist.tile([P, H], F32, tag=f"e1_{t}", name=f"e1_{t}")
               for t in range(NT)]
        rq = [persist.tile([P, H], F32, tag=f"r_{t}", name=f"r_{t}")
              for t in range(NT)]

        def emit_stage_half(t, qd):
            """Wh_ext matmuls + fp16 staging + e1/r + ones col for heads
            [4qd, 4qd+4) at j-tile t."""
            st = stage[t]
            wh_ps = psum.tile([P, 264], F32, tag="bank", bufs=4,
                              name=f"wh{t}_{qd}")
            for kf in range(NKF):
                nc.tensor.matmul(
                    wh_ps[:], xt_sb[kf][:, t * P:(t + 1) * P],
                    wsl(kf, qd * 264, (qd + 1) * 264),
                    start=(kf == 0), stop=(kf == NKF - 1))
            lo = qd * 264
            nc.scalar.activation(st[:, lo:lo + 264], wh_ps[:], AF.Copy)
            nc.scalar.activation(e1q[t][:, 4 * qd:4 * qd + 4],
                                 st[:, lo + 65:lo + 264:66], AF.Exp,
                                 bias=bias_c[:])
            nc.scalar.activation(rq[t][:, 4 * qd:4 * qd + 4],
                                 st[:, lo + 65:lo + 264:66], AF.Exp,
                                 scale=-0.8)
            nc.vector.memset(st[:, lo + 64:lo + 264:66], 1.0)

        # heads 0-3 stages first (with g broadcasts interleaved); heads 4-7
        # stages are emitted between the first heads' attention chains below,
        # filling PE gaps and avoiding a long serial prep phase.
        for t in range(NT):
            if t < H:
                emit_g(t)
            emit_stage_half(t, 0)

        def lhst(h, jt):
            return stage[jt][:, h * 66:h * 66 + 65]

        # ---- layer-1 attention: h outer, quad q inner -------------------
        hT = [persist.tile([P, SL], F16, tag=f"hT{kt}", name=f"hT{kt}")
              for kt in range(NKF)]
        # Wh2 = h @ Wo_ext and the f_src2 row, accumulated one kt (head pair)
        # at a time right after that pair's hT half is written, so only the
        # last kt's small matmuls sit on the pre-AllGather critical path
        wh2acc = persist.tile([P, 264], F32, tag="wh2acc", name="wh2acc")
        fr2acc = persist.tile([1, SL], F32, tag="fr2acc", name="fr2acc")

        def emit_wh2_part(kt):
            o2_ps = psum.tile([P, 264], F32, tag="bank", bufs=4,
                              name=f"o2p{kt}")
            for sub in range(4):
                nc.tensor.matmul(o2_ps[:, sub * 66:(sub + 1) * 66],
                                 hT[kt][:, sub * P:(sub + 1) * P],
                                 woext_all[:, kt * 66:(kt + 1) * 66],
                                 start=True, stop=True)
            fr2_ps = psum.tile([1, SL], F32, tag="bank", bufs=4,
                               name=f"fr2p{kt}")
            nc.tensor.matmul(fr2_ps[:], woext_all[:, kt * 66 + 64:kt * 66 + 65],
                             hT[kt][:], start=True, stop=True)
            if kt == 0:
                nc.scalar.activation(wh2acc[:], o2_ps[:], AF.Copy)
                nc.scalar.activation(fr2acc[:], fr2_ps[:], AF.Copy)
            else:
                nc.vector.tensor_add(wh2acc[:], wh2acc[:], o2_ps[:])
                nc.vector.tensor_add(fr2acc[:], fr2acc[:], fr2_ps[:])

        def emit_masked_quad(uq_name, mq_name, g_t, rq_t, e1q_t, adj_ap,
                             sub=None):
            """TSPs + split Pool/DVE min-mask for one quad. rq_t/e1q_t are
            lists of ([P,1] AP) per k chunk. Returns the masked mq tile."""
            uq = work.tile([P, 4 * SL], F16, tag="uq", bufs=5, name=uq_name)
            mq = work.tile([P, 4 * SL], F16, tag="mq", bufs=5, name=mq_name)
            for k in range(2):
                nc.vector.tensor_scalar(
                    uq[:, k * SL:(k + 1) * SL], g_t, rq_t[k], e1q_t[k],
                    A.max, A.mult)
            nc.gpsimd.tensor_tensor(mq[:, 0:SPOOL], uq[:, 0:SPOOL],
                                    adj_ap[:, 0:SPOOL], A.mult)
            for k in range(2, 4):
                nc.vector.tensor_scalar(
                    uq[:, k * SL:(k + 1) * SL], g_t, rq_t[k], e1q_t[k],
                    A.max, A.mult)
            nc.vector.tensor_tensor(mq[:, SPOOL:4 * SL], uq[:, SPOOL:4 * SL],
                                    adj_ap[:, SPOOL:4 * SL], A.mult)
            return mq

        MM_ORDER = (2, 3, 0, 1)  # DVE-half chunks first: they finish earlier

        for h in range(H):
            if h < 4:
                for t in range(h * 8, h * 8 + 8):
                    emit_stage_half(t, 1)
            if h % 2 == 1 and h > 1:
                emit_wh2_part(h // 2 - 1)
            acc = psum.tile([D + 1, SL], F32, tag="acc", bufs=4,
                            name=f"acc{h}")
            for q in range(NQ):
                jts = [4 * q + k for k in range(4)]
                mq = emit_masked_quad(
                    f"uq{h}_{q}", f"mq{h}_{q}", g_bc[h][:],
                    [rq[jt][:, h:h + 1] for jt in jts],
                    [e1q[jt][:, h:h + 1] for jt in jts], adj_q(q))
                for ki, k in enumerate(MM_ORDER):
                    nc.tensor.matmul(acc[:], lhst(h, jts[k]),
                                     mq[:, k * SL:(k + 1) * SL],
                                     start=(q == 0 and ki == 0),
                                     stop=(q == NQ - 1 and ki == 3))
            # normalization + ELU, written into hT k-tiles (2 heads/tile)
            den_sb = work.tile([1, SL], F32, tag="den", bufs=2,
                               name=f"den{h}")
            nc.scalar.activation(den_sb[:], acc[D:D + 1, :], AF.Copy)
            den_bc = psum.tile([D, SL], F32, tag="bank", bufs=4,
                               name=f"dbc{h}")
            nc.tensor.matmul(den_bc[:], ones1[0:1, 0:D], den_sb[:],
                             start=True, stop=True)
            lnb = work.tile([D, SL], F32, tag="lnb", bufs=2, name=f"lnb{h}")
            nc.scalar.activation(lnb[:], den_bc[:], AF.Ln)
            recb = work.tile([D, SL], F32, tag="recb", bufs=2,
                             name=f"recb{h}")
            nc.scalar.activation(recb[:], lnb[:], AF.Exp, scale=-1.0)
            hsl = hT[h // 2][(h % 2) * D:(h % 2) * D + D, :]
            nc.vector.tensor_mul(hsl, acc[0:D, :], recb[:])
            # ELU in fp16: elu(x) = (x max 0 - 1) + exp(-relu(-x));
            # relu(-x) runs on ACT (scale=-1) to keep DVE light.
            a1 = work.tile([D, SL], F16, tag="a1", bufs=2, name=f"a1{h}")
            nc.scalar.activation(a1[:], hsl, AF.Relu, scale=-1.0)
            texp = work.tile([D, SL], F16, tag="texp", bufs=2,
                             name=f"texp{h}")
            nc.scalar.activation(texp[:], a1[:], AF.Exp, scale=-1.0)
            tlin = work.tile([D, SL], F16, tag="tlin", bufs=2,
                             name=f"tlin{h}")
            nc.vector.tensor_scalar(tlin[:], hsl, 0.0, -1.0, A.max, A.add)
            nc.vector.tensor_add(hsl, texp[:], tlin[:])

        # ---- finish Wh2_ext (last head pair) and stage for the exchange --
        emit_wh2_part(3)
        cc_in = dram.tile([SL, 66], F8, tag="cc_in", name="cc_in")
        cc_full = dram.tile([N, 66], F8, tag="cc_full", addr_space="Shared",
                            name="cc_full")
        p2_sb = work.tile([P, 264], F8, tag="p2", name="p2")
        nc.scalar.activation(p2_sb[:], wh2acc[:], AF.Copy)
        nc.sync.dma_start(
            out=cc_in[:].rearrange("(k p) c -> p k c", k=4),
            in_=p2_sb[:].rearrange("p (k c) -> p k c", k=4))
        bc2_ps = psum.tile([P, SL], F32, tag="bank", bufs=4, name="gbc2")
        nc.tensor.matmul(bc2_ps[:], ones1[0:1, :], fr2acc[:], start=True,
                         stop=True)
        g2 = persist.tile([P, SL], F16, tag="g2", name="g2")
        nc.scalar.activation(g2[:], bc2_ps[:], AF.Exp, scale=0.8)

        nc.gpsimd.collective_compute(
            "AllGather", A.bypass, ins=[cc_in[:]], outs=[cc_full[:]],
            replica_groups=[list(range(N_CORES))])

        # ---- layer 2: two rearranged DMAs load the 8 prep groups (fp8),
        # converted to fp16 per half on ACT (idle right after the gather) ---
        cc_all = persist.tile([P, NQ * 264], F16, tag="cc_all", name="cc_all")
        cc_raw = persist.tile([P, NQ * 264], F8, tag="cc_raw", name="cc_raw")
        HG = NQ // 2
        for gh in range(2):
            sl8 = slice(gh * HG * 264, (gh + 1) * HG * 264)
            nc.sync.dma_start(
                out=cc_raw[:, sl8].rearrange(
                    "p (g k c) -> p g k c", g=HG, k=4),
                in_=cc_full[gh * N // 2:(gh + 1) * N // 2, :].rearrange(
                    "(g k p) c -> p g k c", g=HG, k=4))
            nc.scalar.activation(cc_all[:, sl8], cc_raw[:, sl8], AF.Copy)
        cc_gp = [cc_all[:, g * 264:(g + 1) * 264] for g in range(NQ)]
        e1q2, rq2 = [], []
        for g in range(NQ):
            gp = cc_gp[g]
            e1 = persist.tile([P, 4], F32, tag=f"e2_{g}", name=f"e2_{g}")
            nc.scalar.activation(e1[:], gp[:, 65:264:66], AF.Exp)
            r = persist.tile([P, 4], F32, tag=f"r2_{g}", name=f"r2_{g}")
            nc.scalar.activation(r[:], gp[:, 65:264:66], AF.Exp, scale=-0.8)
            nc.vector.memset(gp[:, 64:264:66], 1.0)
            e1q2.append(e1)
            rq2.append(r)

        # ---- layer-2 attention ------------------------------------------
        acc2 = psum.tile([D + 1, SL], F32, tag="acc", bufs=4, name="acc2")
        for q in range(NQ):
            mq = emit_masked_quad(
                f"uq2_{q}", f"mq2_{q}", g2[:],
                [rq2[q][:, k:k + 1] for k in range(4)],
                [e1q2[q][:, k:k + 1] for k in range(4)], adj_q(q))
            for ki, k in enumerate(MM_ORDER):
                nc.tensor.matmul(acc2[:], cc_gp[q][:, k * 66:k * 66 + 65],
                                 mq[:, k * SL:(k + 1) * SL],
                                 start=(q == 0 and ki == 0),
                                 stop=(q == NQ - 1 and ki == 3))
        den2 = work.tile([1, SL], F32, tag="den", bufs=2, name="den2")
        nc.scalar.activation(den2[:], acc2[D:D + 1, :], AF.Copy)
        dbc2 = psum.tile([D, SL], F32, tag="bank", bufs=4, name="dbc2")
        nc.tensor.matmul(dbc2[:], ones1[0:1, 0:D], den2[:], start=True,
                         stop=True)
        ln2 = work.tile([D, SL], F32, tag="lnb", bufs=2, name="ln2")
        nc.scalar.activation(ln2[:], dbc2[:], AF.Ln)
        rec2 = work.tile([D, SL], F32, tag="recb", bufs=2, name="rec2")
        nc.scalar.activation(rec2[:], ln2[:], AF.Exp, scale=-1.0)
        o2 = persist.tile([D, SL], F32, tag="o2", name="o2")
        nc.vector.tensor_mul(o2[:], acc2[0:D, :], rec2[:])
        # final ELU in fp32 (same relu(-x) decomposition, ACT-heavy)
        a2 = work.tile([D, SL], F32, tag="a2", name="a2")
        nc.scalar.activation(a2[:], o2[:], AF.Relu, scale=-1.0)
        t2exp = work.tile([D, SL], F32, tag="t2exp", name="t2exp")
        nc.scalar.activation(t2exp[:], a2[:], AF.Exp, scale=-1.0)
        t2lin = work.tile([D, SL], F32, tag="t2lin", name="t2lin")
        nc.vector.tensor_scalar(t2lin[:], o2[:], 0.0, -1.0, A.max, A.add)
        fin = persist.tile([D, SL], F32, tag="fin", name="fin")
        nc.vector.tensor_add(fin[:], t2exp[:], t2lin[:])
        nc.sync.dma_start(out=outT[:], in_=fin[:])

    nc.compile()
    return nc


# ---------------------------------------------------------------------------
# host-side driver
# ---------------------------------------------------------------------------

def _prep_inputs(x, adj, W, a, Wo, ao):
    xT16 = np.ascontiguousarray(x.T.astype(np.float16))
    adjT16 = adj.T.astype(np.float16)
    wext = np.empty((F, H * 66), np.float32)
    for h in range(H):
        wext[:, h * 66:h * 66 + D] = W[h]
        wext[:, h * 66 + D] = W[h] @ a[h, :D]
        wext[:, h * 66 + D + 1] = W[h] @ a[h, D:]
    wext = wext.astype(np.float16)
    woext = np.concatenate(
        [Wo, (Wo @ ao[:D])[:, None], (Wo @ ao[D:])[:, None]],
        axis=1).astype(np.float16)

    in_maps = []
    for c in range(N_CORES):
        sl = slice(c * SL, (c + 1) * SL)
        adjq = np.empty((NQ * P, 4 * SL), np.float16)
        for q in range(NQ):
            for k in range(4):
                jt = 4 * q + k
                adjq[q * P:(q + 1) * P, k * SL:(k + 1) * SL] = \
                    adjT16[jt * P:(jt + 1) * P, sl]
        selD = np.zeros((H, H * P), np.float16)
        for h in range(H):
            selD[h, h * P:(h + 1) * P] = 1.0
        in_maps.append({
            "xT": xT16,
            "xS": np.ascontiguousarray(xT16[:, sl]),
            "adjQ": adjq,
            "Wext": wext,
            "Woext": woext,
            "selD": selD,
        })
    return in_maps


def kernel(x, adj, W, a, Wo, ao, cfg):
    x = np.asarray(x, np.float32)
    adj = np.asarray(adj, np.float32)
    W = np.asarray(W, np.float32)
    a = np.asarray(a, np.float32)
    Wo = np.asarray(Wo, np.float32)
    ao = np.asarray(ao, np.float32)

    in_maps = _prep_inputs(x, adj, W, a, Wo, ao)
    if _CACHED.get("nc") is None:
        _CACHED["nc"] = build_kernel()
    res = run_bass_kernel_spmd(_CACHED["nc"], in_maps,
                               core_ids=list(range(N_CORES)))
    out = np.empty((N, D), np.float32)
    for c in range(N_CORES):
        out[c * SL:(c + 1) * SL, :] = res.results[c]["outT"].T
    return out


if __name__ == "__main__":
    import reference as ref_mod
    inputs = {k: np.asarray(v) for k, v in ref_mod.setup_inputs().items()}
    expected = np.asarray(ref_mod.reference(**ref_mod.setup_inputs()))
    got = kernel(**inputs)
    err = np.abs(got - expected).max() / np.abs(expected).max()
    print("rel err:", err)
